# revision 5
# baseline (speedup 1.0000x reference)
"""DescendantMax kernel for Trainium2 (8 NeuronCores, pure data parallel).

Tree structure (hardcoded from the problem spec): balanced 8-ary tree,
DEPTH=6 parent->child levels, BFS node numbering.  Level k starts at
s_k = (8^k - 1) / 7 and has 8^k nodes.  Children of the j-th node of
level k are the 8 consecutive nodes s_{k+1} + 8j ... 8j+8.  So each
level's "gather" is a contiguous reshape, and the whole computation per
batch row is a chain of 8:1 contiguous-group max reductions, each
followed by an elementwise max with the parent level's own input values.

Sharding: x is (64, 299593) f32; batch is sharded across 8 cores
(8 rows per core).

Per-core layout ("row-chunk"): every level-L tensor (all 8 rows) lives
in SBUF as [128, n] where partition q = 16*r + c holds the c-th
contiguous 16th of row r's level-L segment.  Groups of 8 children stay
contiguous inside a partition's free dim at every level down to level
2, so the whole level sweep is plain free-dim 8:1 reduces + elementwise
maxes with NO data movement between levels — the reduce of level L+1 in
this layout IS level L's child-maxima in the same layout.  All DRAM
transfers are large contiguous-per-partition packets (64 KB leaf chunks,
8 KB at level 5, 1 KB at level 4).

The leaf level (8 MB per core) is streamed in 8 free-dim chunks of
[128, 2048] so loads, the 8:1 reduces, and pass-through stores pipeline.
Levels 2 -> 1 -> 0 (tiny) finish in a [8, 64] row-per-partition layout
after one small SBUF->SBUF repack.

Loads trigger on nc.sync's HW DGE queue, stores on nc.scalar's.
"""

import numpy as np

BRANCH = 8
DEPTH = 6
BATCH = 64
N_CORES = 8
ROWS = BATCH // N_CORES  # rows per core
# starts[k] = (8^k - 1) // 7 ; starts[DEPTH+1] == total node count
STARTS = [(BRANCH**k - 1) // (BRANCH - 1) for k in range(DEPTH + 2)]
N_NODES = STARTS[DEPTH + 1]  # 299593
CH = 16  # contiguous chunks per row -> partition q = CH*r + c

_cache: dict = {}


def _build_nc():
    import concourse.bacc as bacc
    import concourse.mybir as mybir
    from concourse.tile import TileContext

    f32 = mybir.dt.float32
    AX = mybir.AxisListType.X

    # Bacc (not raw Bass): its compile() pipeline runs
    # generate_event_semaphores, which splits multi-wait sync_info into
    # EventSemaphore insts — TRN2 allows at most 1 wait per instruction.
    nc = bacc.Bacc(None, target_bir_lowering=False)
    x = nc.dram_tensor("x", [ROWS, N_NODES], f32, kind="ExternalInput")
    out = nc.dram_tensor("out", [ROWS, N_NODES], f32, kind="ExternalOutput")

    def rowchunk(t, lvl):
        """DRAM AP for level lvl of all rows, enumerated (r, c, f) to
        pair 1:1 with a [128, n] row-chunk SBUF tile."""
        a, b = STARTS[lvl], STARTS[lvl + 1]
        return t[:, a:b].rearrange("r (c f) -> r c f", c=CH)

    LEAF_N = BRANCH**DEPTH // CH  # 16384 leaf elems per partition
    NCHUNK = 8
    CW = LEAF_N // NCHUNK  # 2048 leaf columns per pipeline chunk

    with TileContext(nc) as tc:
        with (
            tc.tile_pool(name="big", bufs=4) as big,
            tc.tile_pool(name="tail", bufs=1) as tailp,
        ):
            # level-5 child maxima for all rows, row-chunk layout
            m5 = tailp.tile([128, BRANCH**5 // CH], f32)  # [128, 2048]
            leaf = rowchunk(x, DEPTH)  # [8, 16, 16384]
            leaf_out = rowchunk(out, DEPTH)
            for k in range(NCHUNK):
                t6 = big.tile([128, CW], f32, tag="t6")
                nc.sync.dma_start(
                    out=t6[:, :], in_=leaf[:, :, k * CW : (k + 1) * CW]
                )
                nc.scalar.dma_start(
                    out=leaf_out[:, :, k * CW : (k + 1) * CW], in_=t6[:, :]
                )
                nc.vector.reduce_max(
                    out=m5[:, k * CW // 8 : (k + 1) * CW // 8],
                    in_=t6[:, :].rearrange("q (g e) -> q g e", e=8),
                    axis=AX,
                )

            # levels 5 -> 4 -> 3 -> 2 in row-chunk layout
            prev = m5
            for lvl in (5, 4, 3, 2):
                n = BRANCH**lvl // CH
                xl = tailp.tile([128, n], f32, tag=f"x{lvl}t")
                nc.sync.dma_start(out=xl[:, :], in_=rowchunk(x, lvl))
                o = tailp.tile([128, n], f32, tag=f"o{lvl}t")
                nc.vector.tensor_max(out=o[:, :], in0=prev[:, :], in1=xl[:, :])
                nc.scalar.dma_start(out=rowchunk(out, lvl), in_=o[:, :])
                if lvl > 2:
                    m = tailp.tile([128, n // 8], f32, tag=f"m{lvl - 1}t")
                    nc.vector.reduce_max(
                        out=m[:, :],
                        in_=o[:, :].rearrange("q (g e) -> q g e", e=8),
                        axis=AX,
                    )
                    prev = m
                else:
                    # repack level-2 output [128, 4] -> [8, 64] (one row
                    # per partition; both sides enumerate linearly)
                    t2 = tailp.tile([ROWS, 64], f32)
                    nc.sync.dma_start(out=t2[:, :], in_=o[:, :])
                    m1 = tailp.tile([ROWS, 8], f32)
                    nc.vector.reduce_max(
                        out=m1[:, :],
                        in_=t2[:, :].rearrange("q (g e) -> q g e", e=8),
                        axis=AX,
                    )
                    x1 = tailp.tile([ROWS, 8], f32)
                    nc.sync.dma_start(out=x1[:, :], in_=x[:, 1:9])
                    o1 = tailp.tile([ROWS, 8], f32)
                    nc.vector.tensor_max(
                        out=o1[:, :], in0=m1[:, :], in1=x1[:, :]
                    )
                    nc.scalar.dma_start(out=out[:, 1:9], in_=o1[:, :])
                    m0 = tailp.tile([ROWS, 1], f32)
                    nc.vector.reduce_max(
                        out=m0[:, :],
                        in_=o1[:, :].rearrange("q (g e) -> q g e", e=8),
                        axis=AX,
                    )
                    x0 = tailp.tile([ROWS, 1], f32)
                    nc.sync.dma_start(out=x0[:, :], in_=x[:, 0:1])
                    o0 = tailp.tile([ROWS, 1], f32)
                    nc.vector.tensor_max(
                        out=o0[:, :], in0=m0[:, :], in1=x0[:, :]
                    )
                    nc.scalar.dma_start(out=out[:, 0:1], in_=o0[:, :])
    nc.compile()
    return nc


def _get_nc():
    if "nc" not in _cache:
        _cache["nc"] = _build_nc()
    return _cache["nc"]


def kernel(x, level_parents=None, level_children=None, **_ignored):
    from concourse.bass_utils import run_bass_kernel_spmd

    x = np.ascontiguousarray(np.asarray(x), dtype=np.float32)
    assert x.shape == (BATCH, N_NODES), x.shape

    nc = _get_nc()
    core_ids = list(range(N_CORES))
    in_maps = [
        {"x": x[i * ROWS : (i + 1) * ROWS]} for i in range(N_CORES)
    ]
    res = run_bass_kernel_spmd(nc, in_maps, core_ids)
    return np.concatenate([res.results[i]["out"] for i in range(N_CORES)], axis=0)


# revision 8
# speedup vs baseline: 1.3432x; 1.3432x over previous
"""DescendantMax kernel for Trainium2 (8 NeuronCores, pure data parallel).

Tree structure (hardcoded from the problem spec): balanced 8-ary tree,
DEPTH=6 parent->child levels, BFS node numbering.  Level k starts at
s_k = (8^k - 1) / 7 and has 8^k nodes.  Children of the j-th node of
level k are the 8 consecutive nodes s_{k+1} + 8j ... 8j+8.  So each
level's "gather" is a contiguous reshape, and the whole computation per
batch row is a chain of 8:1 contiguous-group max reductions, each
followed by an elementwise max with the parent level's own input values.

Sharding: x is (64, 299593) f32; batch is sharded across 8 cores
(8 rows per core).

Per-core layout ("row-chunk, c-major"): every level-L tensor (all 8
rows) lives in SBUF as [128, n] where partition q = 8*c + r holds the
c-th contiguous 16th of row r's level-L segment.  Groups of 8 children
stay contiguous inside a partition's free dim at every level down to
level 2, so the level sweep is plain free-dim 8:1 reduces + elementwise
maxes with NO inter-level data movement — the 8:1 reduce of level L+1
in this layout IS level L's child maxima in the same layout.

c-major matters for DMA: the HW descriptor generator stripes a DMA's
packets over the 16 DMA engines by the OUTERMOST access-pattern
dimension, so DRAM APs are emitted as [16(c), 8(r), f] — outer count 16
keeps all 16 engines busy ([8(r), ...] APs only engage 8).

The leaf level (8 MB per core) is streamed in 8 free-dim chunks of
[128, 2048] (8 KB contiguous packets) so loads, reduces, and the
pass-through stores pipeline.  Levels 1 and 0 (tiny) finish in an
[8, 64] row-per-partition layout after 8 small SBUF->SBUF repacks.

Loads trigger on nc.sync's HW DGE queue, stores on nc.scalar's.
"""

import numpy as np

BRANCH = 8
DEPTH = 6
BATCH = 64
N_CORES = 8
ROWS = BATCH // N_CORES  # rows per core
# starts[k] = (8^k - 1) // 7 ; starts[DEPTH+1] == total node count
STARTS = [(BRANCH**k - 1) // (BRANCH - 1) for k in range(DEPTH + 2)]
N_NODES = STARTS[DEPTH + 1]  # 299593
CH = 16  # contiguous chunks per row -> partition q = 8*c + r

_cache: dict = {}


def _build_nc():
    import concourse.bacc as bacc
    import concourse.mybir as mybir
    from concourse.tile import TileContext

    f32 = mybir.dt.float32
    AX = mybir.AxisListType.X

    # Bacc (not raw Bass): its compile() pipeline runs
    # generate_event_semaphores, which splits multi-wait sync_info into
    # EventSemaphore insts — TRN2 allows at most 1 wait per instruction.
    nc = bacc.Bacc(None, target_bir_lowering=False)
    x = nc.dram_tensor("x", [ROWS, N_NODES], f32, kind="ExternalInput")
    out = nc.dram_tensor("out", [ROWS, N_NODES], f32, kind="ExternalOutput")

    def cmajor(t, lvl):
        """DRAM AP for level lvl of all rows, enumerated (c, r, f) to
        pair 1:1 with a [128, n] c-major row-chunk SBUF tile."""
        a, b = STARTS[lvl], STARTS[lvl + 1]
        return t[:, a:b].rearrange("r (c f) -> c r f", c=CH)

    LEAF_N = BRANCH**DEPTH // CH  # 16384 leaf elems per partition
    NCHUNK = 8
    CW = LEAF_N // NCHUNK  # 2048 leaf columns per pipeline chunk

    with TileContext(nc) as tc:
        with (
            tc.tile_pool(name="big", bufs=4) as big,
            tc.tile_pool(name="tail", bufs=1) as tailp,
            tc.tile_pool(name="dram", bufs=1, space="DRAM") as dpool,
        ):
            # level-5 child maxima for all rows, c-major row-chunk layout
            m5 = tailp.tile([128, BRANCH**5 // CH], f32)  # [128, 2048]
            leaf_in = cmajor(x, DEPTH)  # [16, 8, 16384]
            leaf_out = cmajor(out, DEPTH)
            for k in range(NCHUNK):
                t6 = big.tile([128, CW], f32, tag="t6")
                nc.sync.dma_start(
                    out=t6[:, :], in_=leaf_in[:, :, k * CW : (k + 1) * CW]
                )
                nc.scalar.dma_start(
                    out=leaf_out[:, :, k * CW : (k + 1) * CW], in_=t6[:, :]
                )
                nc.vector.reduce_max(
                    out=m5[:, k * CW // 8 : (k + 1) * CW // 8],
                    in_=t6[:, :].rearrange("q (g e) -> q g e", e=8),
                    axis=AX,
                )

            # levels 5 -> 4 -> 3 -> 2 in c-major row-chunk layout
            prev = m5
            for lvl in (5, 4, 3, 2):
                n = BRANCH**lvl // CH
                xl = tailp.tile([128, n], f32, tag=f"x{lvl}t")
                nc.sync.dma_start(out=xl[:, :], in_=cmajor(x, lvl))
                o = tailp.tile([128, n], f32, tag=f"o{lvl}t")
                nc.vector.tensor_max(out=o[:, :], in0=prev[:, :], in1=xl[:, :])
                nc.scalar.dma_start(out=cmajor(out, lvl), in_=o[:, :])
                if lvl > 2:
                    m = tailp.tile([128, n // 8], f32, tag=f"m{lvl - 1}t")
                    nc.vector.reduce_max(
                        out=m[:, :],
                        in_=o[:, :].rearrange("q (g e) -> q g e", e=8),
                        axis=AX,
                    )
                    prev = m

            # repack level-2 output into one-row-per-partition [8, 64] via
            # a Tile-tracked DRAM bounce (strided-partition SBUF APs
            # confuse Tile's dependency tracking)
            d2 = dpool.tile([ROWS, 64], f32)
            nc.scalar.dma_start(
                out=d2[:, :].rearrange("r (c f) -> c r f", c=CH), in_=o[:, :]
            )
            t2 = tailp.tile([ROWS, 64], f32)
            nc.sync.dma_start(out=t2[:, :], in_=d2[:, :])
            # level 1
            m1 = tailp.tile([ROWS, 8], f32)
            nc.vector.reduce_max(
                out=m1[:, :],
                in_=t2[:, :].rearrange("q (g e) -> q g e", e=8),
                axis=AX,
            )
            x1 = tailp.tile([ROWS, 8], f32)
            nc.sync.dma_start(out=x1[:, :], in_=x[:, 1:9])
            o1 = tailp.tile([ROWS, 8], f32)
            nc.vector.tensor_max(out=o1[:, :], in0=m1[:, :], in1=x1[:, :])
            nc.scalar.dma_start(out=out[:, 1:9], in_=o1[:, :])
            # level 0
            m0 = tailp.tile([ROWS, 1], f32)
            nc.vector.reduce_max(
                out=m0[:, :],
                in_=o1[:, :].rearrange("q (g e) -> q g e", e=8),
                axis=AX,
            )
            x0 = tailp.tile([ROWS, 1], f32)
            nc.sync.dma_start(out=x0[:, :], in_=x[:, 0:1])
            o0 = tailp.tile([ROWS, 1], f32)
            nc.vector.tensor_max(out=o0[:, :], in0=m0[:, :], in1=x0[:, :])
            nc.scalar.dma_start(out=out[:, 0:1], in_=o0[:, :])
    nc.compile()
    return nc


def _get_nc():
    if "nc" not in _cache:
        _cache["nc"] = _build_nc()
    return _cache["nc"]


def kernel(x, level_parents=None, level_children=None, **_ignored):
    from concourse.bass_utils import run_bass_kernel_spmd

    x = np.ascontiguousarray(np.asarray(x), dtype=np.float32)
    assert x.shape == (BATCH, N_NODES), x.shape

    nc = _get_nc()
    core_ids = list(range(N_CORES))
    in_maps = [
        {"x": x[i * ROWS : (i + 1) * ROWS]} for i in range(N_CORES)
    ]
    res = run_bass_kernel_spmd(nc, in_maps, core_ids)
    return np.concatenate([res.results[i]["out"] for i in range(N_CORES)], axis=0)
